# revision 40
# baseline (speedup 1.0000x reference)
"""Trainium2 Bass kernel for nn_Attention (B=2, N=2048, C=768, H=12, D=64).

Sharding: 8 cores = 2 batches x 4 head-groups (3 heads each).
Per core: full attention for its (batch, 3 heads) + row-sharded proj
partial output [2048, 768]; host sums the 4 partials per batch (+b_proj).

v2 design (ACT-saturation + p-stationary PV):
  The exp on the ACT engine (12.6M score elems/core at 1 elem/lane/cy
  @1.2GHz) is a hard ~100us floor; everything else is scheduled to hide
  under it.
  - x/w in bf16; x chunks of 256 tokens: PE-transpose -> xT (bf16),
    K matmuls per chunk so the first exp launches ~6us in; Q per
    q-chunk and V per k-pair are interleaved into the attention loop.
  - Scores transposed (sT[k,q] = kT.T @ qT), row-tiled pairs at
    tile_position (0,0)/(64,0) writing DIFFERENT PSUM banks (scA/scB)
    so the two 64-contraction matmuls can overlap. exp in [128,1024]
    ACTIVATEs with scale=0.125 folded in, no max subtraction.
  - PV p-stationary: lhsT = pt[:, q-tile] (128 cols bf16 -> FWL),
    rhs = [v | ones] (N=65): out acc[q,65] accumulates over k; col 64
    is the softmax denominator -> reciprocal_approx_fast on [128,1] +
    per-partition tensor_scalar mult (replaces 40us of 1-lane DVE
    reciprocals + DMA broadcasts in v1).
  - Proj: normalized o written as [q, h0d|h1d] + [q, h2d]; one PE
    transpose stacks h0/h1 on 128 partitions -> proj = K=128 + K=64
    accumulated matmuls per q-tile.
  PSUM: scA/scB 2 banks each, accA/accB 1 each, aux rotation 2 = 8.
"""

import numpy as np
import ml_dtypes

import concourse.bass as bass
import concourse.mybir as mybir
from concourse import bacc, tile
from concourse.bass_utils import run_bass_kernel_spmd
from concourse.masks import make_identity

F32 = mybir.dt.float32
F32R = mybir.dt.float32r
BF16 = mybir.dt.bfloat16
AF = mybir.ActivationFunctionType
ADD = mybir.AluOpType.add
MULT = mybir.AluOpType.mult

B, N, C = 2, 2048, 768
H, D = 12, 64
SCALE = D ** -0.5  # 0.125
NCORES = 8
HPC = 3            # heads per core
NK = N // 128      # 16 k-tiles
NQC = N // 512     # 4 q-chunks
NCH = N // 256     # 8 x chunks
CT = C // 128      # 6 c-tiles


def build_program():
    nc = bacc.Bacc("TRN2", target_bir_lowering=False, debug=False,
                   num_devices=NCORES)
    # host-pre-transposed x, chunk-major so each 256-token chunk DMA is
    # contiguous per partition: xt[ch, p, ct, n'] = x[ch*256 + n', ct*128 + p]
    xt_d = nc.dram_tensor("xt", [NCH, 128, CT * 256], BF16, kind="ExternalInput")
    w_d = nc.dram_tensor("w", [C, 512], BF16, kind="ExternalInput")
    wv_d = nc.dram_tensor("wv", [C, 192], BF16, kind="ExternalInput")
    bq_d = nc.dram_tensor("bq", [128, 4], F32, kind="ExternalInput")
    vb_d = nc.dram_tensor("vb", [128, 192], F32, kind="ExternalInput")
    wp01_d = nc.dram_tensor("wp01", [128, C], F32, kind="ExternalInput")
    wp2_d = nc.dram_tensor("wp2", [64, C], F32, kind="ExternalInput")
    y_d = nc.dram_tensor("y", [N, C], F32, kind="ExternalOutput")

    with tile.TileContext(nc) as tc:
        with (
            tc.tile_pool(name="const", bufs=1) as cpool,
            tc.tile_pool(name="w", bufs=1) as wpool,
            tc.tile_pool(name="xT", bufs=1) as spool,
            tc.tile_pool(name="qk", bufs=1) as qkpool,
            tc.tile_pool(name="v", bufs=1) as vpool,
            tc.tile_pool(name="pt", bufs=12) as ptpool,
            tc.tile_pool(name="o", bufs=1) as opool,
            tc.tile_pool(name="y", bufs=2) as ypool,
            tc.tile_pool(name="r", bufs=4) as rpool,
            tc.tile_pool(name="scA", bufs=1, space="PSUM") as scApool,
            tc.tile_pool(name="acc", bufs=1, space="PSUM") as acpool,
            tc.tile_pool(name="scB", bufs=1, space="PSUM") as scBpool,
            tc.tile_pool(name="aux", bufs=2, space="PSUM") as auxpool,
        ):
            identF = cpool.tile([128, 128], F32)
            make_identity(nc, identF[:])
            bq_sb = cpool.tile([128, 4], F32)
            nc.sync.dma_start(out=bq_sb[:], in_=bq_d[:])
            vb_sb = cpool.tile([128, 192], F32)
            nc.gpsimd.dma_start(out=vb_sb[:], in_=vb_d[:])

            w_r = wpool.tile([128, CT, 512], BF16)
            nc.gpsimd.dma_start(
                out=w_r[:], in_=w_d.ap().rearrange("(t p) m -> p t m", p=128))
            wv_r = wpool.tile([128, CT, 192], BF16)
            nc.gpsimd.dma_start(
                out=wv_r[:], in_=wv_d.ap().rearrange("(t p) m -> p t m", p=128))
            wp01_f = wpool.tile([128, C], F32)
            nc.gpsimd.dma_start(out=wp01_f[:], in_=wp01_d[:])
            wp01_r = wpool.tile([128, C], F32R)
            nc.vector.tensor_copy(wp01_r[:], wp01_f[:])
            wp2_f = wpool.tile([64, C], F32)
            nc.gpsimd.dma_start(out=wp2_f[:], in_=wp2_d[:])
            wp2_r = wpool.tile([64, C], F32R)
            nc.vector.tensor_copy(wp2_r[:], wp2_f[:])

            xT = spool.tile([128, NCH, CT, 256], BF16, tag="xT", name="xT")
            qT01 = qkpool.tile([128, N], BF16, tag="qT01", name="qT01")
            kT01 = qkpool.tile([128, N], BF16, tag="kT01", name="kT01")
            qT22 = qkpool.tile([128, N], BF16, tag="qT22", name="qT22")
            kT22 = qkpool.tile([128, N], BF16, tag="kT22", name="kT22")
            # v layout: [keys, k-tile, h0 d(64)+1 | h1 d+1 | h2 d+1] (3*65)
            vall = vpool.tile([128, NK, 195], BF16, tag="vall", name="vall")
            for h in range(HPC):
                nc.vector.memset(vall[:, :, h * 65 + 64:h * 65 + 65], 1.0)

            accA = acpool.tile([128, 512], F32, tag="accA", name="accA")
            accB = acpool.tile([128, 512], F32, tag="accB", name="accB")

            def chain_slice(h, qj):
                """PSUM (tile, col) of PV accumulator for chain (head, qj)."""
                if h == 0:
                    return accA, qj * 65
                if h == 1:
                    return (accA, 260 + qj * 65) if qj < 2 else (accB, (qj - 2) * 65)
                return accB, 130 + qj * 65

            def chunk_dma(ch):
                nc.sync.dma_start(
                    out=xT[:, ch, :, :].rearrange("p ct n -> p (ct n)"),
                    in_=xt_d[ch, :, :])

            def kq_block(ch, t, dst, bcol):
                """One 256-token slab of one qkv weight tile t (6 matmuls)."""
                ns = slice(ch * 256, (ch + 1) * 256)
                kp = auxpool.tile([128, 512], F32, tag="aux",
                                  name=f"kq{ch}_{t}")
                for ct in range(CT):
                    nc.tensor.matmul(kp[:, 0:256],
                                     w_r[:, ct, t * 128:(t + 1) * 128],
                                     xT[:, ch, ct, :], start=(ct == 0),
                                     stop=(ct == CT - 1))
                nc.vector.tensor_scalar(dst[:, ns], kp[:, 0:256],
                                        bq_sb[:, bcol:bcol + 1], None, ADD)

            def v_block(k0):
                """v for k-tiles k0, k0+1 -> vall (with bias add)."""
                vp = auxpool.tile([128, 512], F32, tag="aux", name=f"vp{k0}")
                for i in range(2):
                    k = k0 + i
                    ch, off = k // 2, (k % 2) * 128
                    for ct in range(CT):
                        nc.tensor.matmul(vp[:, i * 256:i * 256 + 192],
                                         xT[:, ch, ct, off:off + 128],
                                         wv_r[:, ct, :],
                                         start=(ct == 0), stop=(ct == CT - 1))
                for i in range(2):
                    src = vp[:, i * 256:i * 256 + 192].rearrange(
                        "p (h x) -> p h x", h=3)
                    dst = vall[:, k0 + i, :].rearrange(
                        "p (h x) -> p h x", h=3)[:, :, 0:64]
                    nc.vector.tensor_add(dst, src,
                                         vb_sb[:].rearrange("p (h x) -> p h x", h=3))

            def scores_pair(qc, j):
                """Heads 0/1, k-tiles 2j, 2j+1 -> exp'd pt tiles."""
                qs = slice(qc * 512, (qc + 1) * 512)
                scA = scApool.tile([128, 1024], F32, tag="scA", name="scA")
                scB = scBpool.tile([128, 1024], F32, tag="scB", name="scB")
                for kk in range(2):
                    ks = slice((2 * j + kk) * 128, (2 * j + kk + 1) * 128)
                    nc.tensor.matmul(scA[:, kk * 512:(kk + 1) * 512],
                                     kT01[0:64, ks], qT01[0:64, qs],
                                     start=True, stop=True)
                    nc.tensor.matmul(scB[:, kk * 512:(kk + 1) * 512],
                                     kT01[64:128, ks], qT01[64:128, qs],
                                     start=True, stop=True, tile_position=(64, 0))
                ptA = ptpool.tile([128, 1024], BF16, tag="ptA", name="ptA")
                ptB = ptpool.tile([128, 1024], BF16, tag="ptB", name="ptB")
                nc.scalar.activation(ptA[:], scA[:], AF.Exp, scale=SCALE)
                nc.scalar.activation(ptB[:], scB[:], AF.Exp, scale=SCALE)
                return ptA, ptB

            def scores_h2(qc, i):
                """Head 2: scA gets k=4i,4i+2 (rt0); scB k=4i+1,4i+3 (rt1)."""
                qs = slice(qc * 512, (qc + 1) * 512)
                scA = scApool.tile([128, 1024], F32, tag="scA", name="scA")
                scB = scBpool.tile([128, 1024], F32, tag="scB", name="scB")
                for kk in range(2):
                    kA = 4 * i + 2 * kk
                    kB = kA + 1
                    nc.tensor.matmul(scA[:, kk * 512:(kk + 1) * 512],
                                     kT22[0:64, kA * 128:(kA + 1) * 128],
                                     qT22[0:64, qs], start=True, stop=True)
                    nc.tensor.matmul(scB[:, kk * 512:(kk + 1) * 512],
                                     kT22[64:128, kB * 128:(kB + 1) * 128],
                                     qT22[64:128, qs], start=True, stop=True,
                                     tile_position=(64, 0))
                ptA = ptpool.tile([128, 1024], BF16, tag="ptA", name="ptA2")
                ptB = ptpool.tile([128, 1024], BF16, tag="ptB", name="ptB2")
                nc.scalar.activation(ptA[:], scA[:], AF.Exp, scale=SCALE)
                nc.scalar.activation(ptB[:], scB[:], AF.Exp, scale=SCALE)
                return ptA, ptB

            def pv_pair_piece(ptsA, ptsB, piece):
                """2 chains (= sequential PSUM accum groups), 32 matmuls."""
                chains = [(0, 0), (0, 1), (0, 2), (0, 3),
                          (1, 0), (1, 1), (1, 2), (1, 3)]
                for h, qj in chains[2 * piece:2 * piece + 2]:
                    pts = ptsA if h == 0 else ptsB
                    acc, c0 = chain_slice(h, qj)
                    for k in range(NK):
                        pt = pts[k // 2]
                        q0 = (k % 2) * 512 + qj * 128
                        nc.tensor.matmul(acc[:, c0:c0 + 65],
                                         pt[:, q0:q0 + 128],
                                         vall[:, k, h * 65:(h + 1) * 65],
                                         start=(k == 0), stop=(k == NK - 1))

            def pv_h2_chain(hA, hB, qj):
                """One h2 chain (16 matmuls)."""
                acc, c0 = chain_slice(2, qj)
                seq = []
                for i in range(4):
                    seq += [(hA[i], 0, 4 * i), (hA[i], 1, 4 * i + 2),
                            (hB[i], 0, 4 * i + 1), (hB[i], 1, 4 * i + 3)]
                for n, (pt, kk, k) in enumerate(seq):
                    q0 = kk * 512 + qj * 128
                    nc.tensor.matmul(acc[:, c0:c0 + 65], pt[:, q0:q0 + 128],
                                     vall[:, k, 130:195],
                                     start=(n == 0), stop=(n == NK - 1))

            o01s = [None] * 4
            o2s = [None] * 4

            def norm_qj(qc, qj):
                """DVE: acc -> normalized o for one q-tile (3 chains)."""
                o01 = opool.tile([128, 128], F32, tag=f"o01_{qj}",
                                 name=f"o01_{qc}_{qj}")
                o2 = opool.tile([128, 64], F32, tag=f"o2_{qj}",
                                name=f"o2_{qc}_{qj}")
                for h in range(HPC):
                    acc, c0 = chain_slice(h, qj)
                    r = rpool.tile([128, 1], F32, tag="r", name="r")
                    with nc.allow_low_precision(reason="softmax denom recip"):
                        nc.vector.reciprocal_approx_fast(
                            r[:], acc[:, c0 + 64:c0 + 65])
                    dst = o01[:, h * 64:(h + 1) * 64] if h < 2 else o2[:]
                    nc.vector.tensor_scalar(dst, acc[:, c0:c0 + 64],
                                            r[:], None, MULT)
                o01s[qj] = o01
                o2s[qj] = o2

            oT01s = [None] * 4
            oT2s = [None] * 4

            def proj_t(qc, qj):
                """PE transposes + DVE copies for one q-tile (stage 1)."""
                tT = auxpool.tile([128, 512], F32, tag="aux",
                                  name=f"oT{qc}_{qj}")
                nc.tensor.transpose(tT[:, 0:128], o01s[qj][:], identF[:])
                nc.tensor.transpose(tT[0:64, 128:256], o2s[qj][:], identF[:])
                oT01 = opool.tile([128, 128], F32R, tag="oT01", bufs=4,
                                  name="oT01")
                oT2 = opool.tile([64, 128], F32R, tag="oT2", bufs=4,
                                 name="oT2")
                nc.vector.tensor_copy(oT01[:], tT[:, 0:128])
                nc.vector.tensor_copy(oT2[:], tT[0:64, 128:256])
                oT01s[qj] = oT01
                oT2s[qj] = oT2

            def proj_p(qc, qj):
                """proj matmuls + y out for one q-tile (stage 2)."""
                qrows = slice(qc * 512 + qj * 128, qc * 512 + (qj + 1) * 128)
                oT01, oT2 = oT01s[qj], oT2s[qj]
                ya = auxpool.tile([128, 512], F32, tag="aux",
                                  name=f"ya{qc}_{qj}")
                nc.tensor.matmul(ya[:], oT01[:], wp01_r[:, 0:512],
                                 start=True, stop=False)
                nc.tensor.matmul(ya[:], oT2[:], wp2_r[:, 0:512],
                                 start=False, stop=True)
                yb = auxpool.tile([128, 512], F32, tag="aux",
                                  name=f"yb{qc}_{qj}")
                nc.tensor.matmul(yb[:, 0:256], oT01[:], wp01_r[:, 512:768],
                                 start=True, stop=False)
                nc.tensor.matmul(yb[:, 0:256], oT2[:], wp2_r[:, 512:768],
                                 start=False, stop=True)
                y_sb = ypool.tile([128, C], F32, tag="y", name="ysb")
                nc.vector.tensor_copy(y_sb[:, 0:512], ya[:])
                nc.vector.tensor_copy(y_sb[:, 512:768], yb[:, 0:256])
                nc.sync.dma_start(out=y_d[qrows, :], in_=y_sb[:])

            # ---------------- schedule ----------------
            # per-round filler work keeps PE bursts small so the exp
            # stream on ACT never starves; all pieces are ~1us or less.
            def k1(ch):
                kq_block(ch, 1, kT01, 1)

            def k3(ch):
                kq_block(ch, 3, kT22, 3)

            def q0(ch):
                kq_block(ch, 0, qT01, 0)

            def q2(ch):
                kq_block(ch, 2, qT22, 2)

            def run_fillers(items):
                for f in items:
                    f()

            chunk_dma(0)
            chunk_dma(1)
            for ch in range(2, NCH):
                # eager bulk x DMAs on the (otherwise idle) gpsimd queue,
                # issued before any xT reader so nothing blocks the queue
                nc.gpsimd.dma_start(
                    out=xT[:, ch, :, :].rearrange("p ct n -> p (ct n)"),
                    in_=xt_d[ch, :, :])
            k1(0)
            q0(0)
            q0(1)
            # Filler plan per (qc, round 0..11). Deadlines: k1(ch) before
            # pair round ch; k3(2i),k3(2i+1) before h2 round i; q0/q2
            # slabs before their qc's pair/h2 rounds; v before pv pieces;
            # h2-chains 2,3 + norm of qc-1 before this qc's PV overwrites
            # acc (rounds 0-1).  pv pieces per qc: pair 0..3 at h2 rounds,
            # h2 chains 0,1 at qc end, chains 2,3 early next qc.
            prev = {"pts": None}

            def qc0_fill(j):
                if j < 7:
                    k1(j + 1)
                v_block(2 * j)
                if j == 4:
                    k3(0)
                if j == 5:
                    k3(1)
                if j == 6:
                    k3(2)
                    k3(3)
                if j == 7:
                    q2(0)
                    q2(1)

            for qc in range(NQC):
                ptsA, ptsB, hA, hB = [], [], [], []
                for j in range(8):
                    a, b = scores_pair(qc, j)
                    ptsA.append(a)
                    ptsB.append(b)
                    if qc == 0:
                        qc0_fill(j)
                    else:
                        pA0, pB0, hA0, hB0 = prev["pts"]
                        if j == 0:
                            pv_h2_chain(hA0, hB0, 2)
                            norm_qj(qc - 1, 0)
                            norm_qj(qc - 1, 1)
                        elif j == 1:
                            pv_h2_chain(hA0, hB0, 3)
                            norm_qj(qc - 1, 2)
                            norm_qj(qc - 1, 3)
                            proj_t(qc - 1, 0)
                        elif j in (2, 3, 4):
                            proj_p(qc - 1, j - 2)
                            proj_t(qc - 1, j - 1)
                        elif j == 5:
                            proj_p(qc - 1, 3)
                        elif j == 6:
                            q2(2 * qc)
                        elif j == 7:
                            q2(2 * qc + 1)
                for i in range(4):
                    a, b = scores_h2(qc, i)
                    hA.append(a)
                    hB.append(b)
                    if qc == 0:
                        if i == 0:
                            k3(4)
                            k3(5)
                        elif i == 1:
                            k3(6)
                            k3(7)
                            pv_pair_piece(ptsA, ptsB, 0)
                        elif i == 2:
                            pv_pair_piece(ptsA, ptsB, 1)
                            pv_pair_piece(ptsA, ptsB, 2)
                        else:
                            pv_pair_piece(ptsA, ptsB, 3)
                            q0(2)
                            q0(3)
                    else:
                        pv_pair_piece(ptsA, ptsB, i)
                        if qc < NQC - 1:
                            if i == 2:
                                q0(2 * qc + 2)
                            elif i == 3:
                                q0(2 * qc + 3)
                pv_h2_chain(hA, hB, 0)
                pv_h2_chain(hA, hB, 1)
                prev["pts"] = (ptsA, ptsB, hA, hB)
            # tail: per-qj pipeline to keep PE dense and DVE off the
            # critical path (norm(qj) ready one step before its T/P)
            pA0, pB0, hA0, hB0 = prev["pts"]
            qn = NQC - 1
            pv_h2_chain(hA0, hB0, 2)
            norm_qj(qn, 0)
            norm_qj(qn, 1)
            proj_t(qn, 0)
            pv_h2_chain(hA0, hB0, 3)
            norm_qj(qn, 2)
            norm_qj(qn, 3)
            proj_t(qn, 1)
            proj_p(qn, 0)
            proj_t(qn, 2)
            proj_p(qn, 1)
            proj_t(qn, 3)
            proj_p(qn, 2)
            proj_p(qn, 3)

    nc.compile()
    return nc


def make_in_maps(x, w_qkv, b_qkv, w_proj):
    """Per-core input dicts. Core c: batch c//4, heads 3*(c%4)+[0..2]."""
    x = np.asarray(x, np.float32)
    w_qkv = np.asarray(w_qkv, np.float32)
    b_qkv = np.asarray(b_qkv, np.float32)
    w_proj = np.asarray(w_proj, np.float32)
    bf = ml_dtypes.bfloat16
    q = lambda h: w_qkv[:, h * 64:(h + 1) * 64]
    k = lambda h: w_qkv[:, C + h * 64:C + (h + 1) * 64]
    v = lambda h: w_qkv[:, 2 * C + h * 64:2 * C + (h + 1) * 64]
    qb = lambda h: b_qkv[h * 64:(h + 1) * 64]
    kb = lambda h: b_qkv[C + h * 64:C + (h + 1) * 64]
    vb = lambda h: b_qkv[2 * C + h * 64:2 * C + (h + 1) * 64]
    in_maps = []
    for c in range(NCORES):
        b = c // 4
        h0 = 3 * (c % 4)
        h1, h2 = h0 + 1, h0 + 2
        w_pack = np.concatenate(
            [q(h0), q(h1), k(h0), k(h1), q(h2), q(h2), k(h2), k(h2)], axis=1)
        bias = np.concatenate(
            [qb(h0), qb(h1), kb(h0), kb(h1), qb(h2), qb(h2), kb(h2), kb(h2)])
        bq_pack = bias.reshape(4, 128).T.copy()
        wv_pack = np.concatenate([v(h0), v(h1), v(h2)], axis=1)
        vb_pack = np.broadcast_to(
            np.concatenate([vb(h0), vb(h1), vb(h2)]), (128, 192))
        wp01 = np.concatenate(
            [w_proj[h0 * 64:(h0 + 1) * 64], w_proj[h1 * 64:(h1 + 1) * 64]],
            axis=0)
        wp2 = w_proj[h2 * 64:(h2 + 1) * 64]
        # host transpose, chunk-major: xt[ch, p, ct*256+n'] = x[b][ch*256+n', ct*128+p]
        xt = np.ascontiguousarray(
            x[b].reshape(NCH, 256, CT, 128).transpose(0, 3, 2, 1).reshape(
                NCH, 128, CT * 256)).astype(bf)
        in_maps.append({
            "xt": xt,
            "w": np.ascontiguousarray(w_pack).astype(bf),
            "wv": np.ascontiguousarray(wv_pack).astype(bf),
            "bq": np.ascontiguousarray(bq_pack, np.float32),
            "vb": np.ascontiguousarray(vb_pack, np.float32),
            "wp01": np.ascontiguousarray(wp01, np.float32),
            "wp2": np.ascontiguousarray(wp2, np.float32),
        })
    return in_maps


_NC_CACHE = []


def _get_program():
    if not _NC_CACHE:
        _NC_CACHE.append(build_program())
    return _NC_CACHE[0]


def run(inputs, trace=False, **kw):
    nc = _get_program()
    in_maps = make_in_maps(inputs["x"], inputs["w_qkv"], inputs["b_qkv"],
                           inputs["w_proj"])
    res = run_bass_kernel_spmd(nc, in_maps, list(range(NCORES)), trace=trace, **kw)
    b_proj = np.asarray(inputs["b_proj"], np.float32)
    out = np.zeros((B, N, C), np.float32)
    for c in range(NCORES):
        out[c // 4] += res.results[c]["y"]
    out += b_proj[None, None, :]
    return out.astype(np.float32), res


def kernel(**inputs):
    out, _ = run(inputs)
    return out


# revision 41
# speedup vs baseline: 1.1368x; 1.1368x over previous
"""Trainium2 Bass kernel for nn_Attention (B=2, N=2048, C=768, H=12, D=64).

Sharding: 8 cores = 2 batches x 4 head-groups (3 heads each).
Per core: full attention for its (batch, 3 heads) + row-sharded proj
partial output [2048, 768]; host sums the 4 partials per batch (+b_proj).

v2 design (ACT-saturation + p-stationary PV):
  The exp on the ACT engine (12.6M score elems/core at 1 elem/lane/cy
  @1.2GHz) is a hard ~100us floor; everything else is scheduled to hide
  under it.
  - x/w in bf16; x chunks of 256 tokens: PE-transpose -> xT (bf16),
    K matmuls per chunk so the first exp launches ~6us in; Q per
    q-chunk and V per k-pair are interleaved into the attention loop.
  - Scores transposed (sT[k,q] = kT.T @ qT), row-tiled pairs at
    tile_position (0,0)/(64,0) writing DIFFERENT PSUM banks (scA/scB)
    so the two 64-contraction matmuls can overlap. exp in [128,1024]
    ACTIVATEs with scale=0.125 folded in, no max subtraction.
  - PV p-stationary: lhsT = pt[:, q-tile] (128 cols bf16 -> FWL),
    rhs = [v | ones] (N=65): out acc[q,65] accumulates over k; col 64
    is the softmax denominator -> reciprocal_approx_fast on [128,1] +
    per-partition tensor_scalar mult (replaces 40us of 1-lane DVE
    reciprocals + DMA broadcasts in v1).
  - Proj: normalized o written as [q, h0d|h1d] + [q, h2d]; one PE
    transpose stacks h0/h1 on 128 partitions -> proj = K=128 + K=64
    accumulated matmuls per q-tile.
  PSUM: scA/scB 2 banks each, accA/accB 1 each, aux rotation 2 = 8.
"""

import numpy as np
import ml_dtypes

import concourse.bass as bass
import concourse.mybir as mybir
from concourse import bacc, tile
from concourse.bass_utils import run_bass_kernel_spmd
from concourse.masks import make_identity

F32 = mybir.dt.float32
F32R = mybir.dt.float32r
BF16 = mybir.dt.bfloat16
AF = mybir.ActivationFunctionType
ADD = mybir.AluOpType.add
MULT = mybir.AluOpType.mult

B, N, C = 2, 2048, 768
H, D = 12, 64
SCALE = D ** -0.5  # 0.125
NCORES = 8
HPC = 3            # heads per core
NK = N // 128      # 16 k-tiles
NQC = N // 512     # 4 q-chunks
NCH = N // 256     # 8 x chunks
CT = C // 128      # 6 c-tiles


def build_program():
    nc = bacc.Bacc("TRN2", target_bir_lowering=False, debug=False,
                   num_devices=NCORES)
    # host-pre-transposed x, chunk-major so each 256-token chunk DMA is
    # contiguous per partition: xt[ch, p, ct, n'] = x[ch*256 + n', ct*128 + p]
    xt_d = nc.dram_tensor("xt", [NCH, 128, CT * 256], BF16, kind="ExternalInput")
    w_d = nc.dram_tensor("w", [C, 512], BF16, kind="ExternalInput")
    wv_d = nc.dram_tensor("wv", [C, 192], BF16, kind="ExternalInput")
    bq_d = nc.dram_tensor("bq", [128, 4], F32, kind="ExternalInput")
    vb_d = nc.dram_tensor("vb", [128, 192], F32, kind="ExternalInput")
    wp01_d = nc.dram_tensor("wp01", [128, C], F32, kind="ExternalInput")
    wp2_d = nc.dram_tensor("wp2", [64, C], F32, kind="ExternalInput")
    y_d = nc.dram_tensor("y", [N, C], F32, kind="ExternalOutput")

    with tile.TileContext(nc) as tc:
        with (
            tc.tile_pool(name="const", bufs=1) as cpool,
            tc.tile_pool(name="w", bufs=1) as wpool,
            tc.tile_pool(name="xT", bufs=1) as spool,
            tc.tile_pool(name="qk", bufs=1) as qkpool,
            tc.tile_pool(name="v", bufs=1) as vpool,
            tc.tile_pool(name="pt", bufs=12) as ptpool,
            tc.tile_pool(name="o", bufs=1) as opool,
            tc.tile_pool(name="y", bufs=2) as ypool,
            tc.tile_pool(name="r", bufs=4) as rpool,
            tc.tile_pool(name="sc", bufs=1, space="PSUM") as scpool,
            tc.tile_pool(name="acc", bufs=1, space="PSUM") as acpool,
            tc.tile_pool(name="aux", bufs=2, space="PSUM") as auxpool,
        ):
            xT = spool.tile([128, NCH, CT, 256], BF16, tag="xT", name="xT")
            w_r = wpool.tile([128, CT, 512], BF16)
            # critical-path DMAs first: w (gpsimd) and x chunks 0-1 (sync)
            nc.gpsimd.dma_start(
                out=w_r[:], in_=w_d.ap().rearrange("(t p) m -> p t m", p=128))
            bq_sb = cpool.tile([128, 4], F32)
            nc.sync.dma_start(out=bq_sb[:], in_=bq_d[:])
            identF = cpool.tile([128, 128], F32)
            make_identity(nc, identF[:])
            wv_r = wpool.tile([128, CT, 192], BF16)
            vb_sb = cpool.tile([128, 192], F32)
            wp01_f = wpool.tile([128, C], F32)
            wp01_r = wpool.tile([128, C], F32R)
            wp2_f = wpool.tile([64, C], F32)
            wp2_r = wpool.tile([64, C], F32R)

            def bulk_dmas():
                nc.gpsimd.dma_start(
                    out=wv_r[:],
                    in_=wv_d.ap().rearrange("(t p) m -> p t m", p=128))
                for ch in range(2, NCH):
                    nc.gpsimd.dma_start(
                        out=xT[:, ch, :, :].rearrange("p ct n -> p (ct n)"),
                        in_=xt_d[ch, :, :])
                nc.gpsimd.dma_start(out=vb_sb[:], in_=vb_d[:])
                nc.gpsimd.dma_start(out=wp01_f[:], in_=wp01_d[:])
                nc.vector.tensor_copy(wp01_r[:], wp01_f[:])
                nc.gpsimd.dma_start(out=wp2_f[:], in_=wp2_d[:])
                nc.vector.tensor_copy(wp2_r[:], wp2_f[:])
            qT01 = qkpool.tile([128, N], BF16, tag="qT01", name="qT01")
            kT01 = qkpool.tile([128, N], BF16, tag="kT01", name="kT01")
            qT22 = qkpool.tile([128, N], BF16, tag="qT22", name="qT22")
            kT22 = qkpool.tile([128, N], BF16, tag="kT22", name="kT22")
            # v layout: [keys, k-tile, h0 d(64)+1 | h1 d+1 | h2 d+1] (3*65)
            vall = vpool.tile([128, NK, 195], BF16, tag="vall", name="vall")
            for h in range(HPC):
                nc.vector.memset(vall[:, :, h * 65 + 64:h * 65 + 65], 1.0)

            accA = acpool.tile([128, 512], F32, tag="accA", name="accA")
            accB = acpool.tile([128, 512], F32, tag="accB", name="accB")

            def chain_slice(h, qj):
                """PSUM (tile, col) of PV accumulator for chain (head, qj)."""
                if h == 0:
                    return accA, qj * 65
                if h == 1:
                    return (accA, 260 + qj * 65) if qj < 2 else (accB, (qj - 2) * 65)
                return accB, 130 + qj * 65

            def chunk_dma(ch):
                nc.sync.dma_start(
                    out=xT[:, ch, :, :].rearrange("p ct n -> p (ct n)"),
                    in_=xt_d[ch, :, :])

            def kq_block(ch, t, dst, bcol):
                """One 256-token slab of one qkv weight tile t (6 matmuls)."""
                ns = slice(ch * 256, (ch + 1) * 256)
                kp = auxpool.tile([128, 512], F32, tag="aux",
                                  name=f"kq{ch}_{t}")
                for ct in range(CT):
                    nc.tensor.matmul(kp[:, 0:256],
                                     w_r[:, ct, t * 128:(t + 1) * 128],
                                     xT[:, ch, ct, :], start=(ct == 0),
                                     stop=(ct == CT - 1))
                nc.vector.tensor_scalar(dst[:, ns], kp[:, 0:256],
                                        bq_sb[:, bcol:bcol + 1], None, ADD)

            def v_block(k0):
                """v for k-tiles k0, k0+1 -> vall (with bias add)."""
                vp = auxpool.tile([128, 512], F32, tag="aux", name=f"vp{k0}")
                for i in range(2):
                    k = k0 + i
                    ch, off = k // 2, (k % 2) * 128
                    for ct in range(CT):
                        nc.tensor.matmul(vp[:, i * 256:i * 256 + 192],
                                         xT[:, ch, ct, off:off + 128],
                                         wv_r[:, ct, :],
                                         start=(ct == 0), stop=(ct == CT - 1))
                for i in range(2):
                    src = vp[:, i * 256:i * 256 + 192].rearrange(
                        "p (h x) -> p h x", h=3)
                    dst = vall[:, k0 + i, :].rearrange(
                        "p (h x) -> p h x", h=3)[:, :, 0:64]
                    nc.vector.tensor_add(dst, src,
                                         vb_sb[:].rearrange("p (h x) -> p h x", h=3))

            def scores_pair(qc, j):
                """Heads 0/1, k-tiles 2j, 2j+1 -> exp'd pt tiles."""
                qs = slice(qc * 512, (qc + 1) * 512)
                scA = scpool.tile([128, 1024], F32, tag="scA", name="scA")
                scB = scpool.tile([128, 1024], F32, tag="scB", name="scB")
                for kk in range(2):
                    ks = slice((2 * j + kk) * 128, (2 * j + kk + 1) * 128)
                    nc.tensor.matmul(scA[:, kk * 512:(kk + 1) * 512],
                                     kT01[0:64, ks], qT01[0:64, qs],
                                     start=True, stop=True)
                    nc.tensor.matmul(scB[:, kk * 512:(kk + 1) * 512],
                                     kT01[64:128, ks], qT01[64:128, qs],
                                     start=True, stop=True, tile_position=(64, 0))
                ptA = ptpool.tile([128, 1024], BF16, tag="ptA", name="ptA")
                ptB = ptpool.tile([128, 1024], BF16, tag="ptB", name="ptB")
                nc.scalar.activation(ptA[:], scA[:], AF.Exp, scale=SCALE)
                nc.scalar.activation(ptB[:], scB[:], AF.Exp, scale=SCALE)
                return ptA, ptB

            def scores_h2(qc, i):
                """Head 2: scA gets k=4i,4i+2 (rt0); scB k=4i+1,4i+3 (rt1)."""
                qs = slice(qc * 512, (qc + 1) * 512)
                scA = scpool.tile([128, 1024], F32, tag="scA", name="scA")
                scB = scpool.tile([128, 1024], F32, tag="scB", name="scB")
                for kk in range(2):
                    kA = 4 * i + 2 * kk
                    kB = kA + 1
                    nc.tensor.matmul(scA[:, kk * 512:(kk + 1) * 512],
                                     kT22[0:64, kA * 128:(kA + 1) * 128],
                                     qT22[0:64, qs], start=True, stop=True)
                    nc.tensor.matmul(scB[:, kk * 512:(kk + 1) * 512],
                                     kT22[64:128, kB * 128:(kB + 1) * 128],
                                     qT22[64:128, qs], start=True, stop=True,
                                     tile_position=(64, 0))
                ptA = ptpool.tile([128, 1024], BF16, tag="ptA", name="ptA2")
                ptB = ptpool.tile([128, 1024], BF16, tag="ptB", name="ptB2")
                nc.scalar.activation(ptA[:], scA[:], AF.Exp, scale=SCALE)
                nc.scalar.activation(ptB[:], scB[:], AF.Exp, scale=SCALE)
                return ptA, ptB

            def pv_pair_piece(ptsA, ptsB, piece):
                """2 chains (= sequential PSUM accum groups), 32 matmuls."""
                chains = [(0, 0), (0, 1), (0, 2), (0, 3),
                          (1, 0), (1, 1), (1, 2), (1, 3)]
                for h, qj in chains[2 * piece:2 * piece + 2]:
                    pts = ptsA if h == 0 else ptsB
                    acc, c0 = chain_slice(h, qj)
                    for k in range(NK):
                        pt = pts[k // 2]
                        q0 = (k % 2) * 512 + qj * 128
                        nc.tensor.matmul(acc[:, c0:c0 + 65],
                                         pt[:, q0:q0 + 128],
                                         vall[:, k, h * 65:(h + 1) * 65],
                                         start=(k == 0), stop=(k == NK - 1))

            def pv_h2_chain(hA, hB, qj):
                """One h2 chain (16 matmuls)."""
                acc, c0 = chain_slice(2, qj)
                seq = []
                for i in range(4):
                    seq += [(hA[i], 0, 4 * i), (hA[i], 1, 4 * i + 2),
                            (hB[i], 0, 4 * i + 1), (hB[i], 1, 4 * i + 3)]
                for n, (pt, kk, k) in enumerate(seq):
                    q0 = kk * 512 + qj * 128
                    nc.tensor.matmul(acc[:, c0:c0 + 65], pt[:, q0:q0 + 128],
                                     vall[:, k, 130:195],
                                     start=(n == 0), stop=(n == NK - 1))

            o01s = [None] * 4
            o2s = [None] * 4

            def norm_qj(qc, qj):
                """DVE: acc -> normalized o for one q-tile (3 chains)."""
                o01 = opool.tile([128, 128], F32, tag=f"o01_{qj}",
                                 name=f"o01_{qc}_{qj}")
                o2 = opool.tile([128, 64], F32, tag=f"o2_{qj}",
                                name=f"o2_{qc}_{qj}")
                for h in range(HPC):
                    acc, c0 = chain_slice(h, qj)
                    r = rpool.tile([128, 1], F32, tag="r", name="r")
                    with nc.allow_low_precision(reason="softmax denom recip"):
                        nc.vector.reciprocal_approx_fast(
                            r[:], acc[:, c0 + 64:c0 + 65])
                    dst = o01[:, h * 64:(h + 1) * 64] if h < 2 else o2[:]
                    nc.vector.tensor_scalar(dst, acc[:, c0:c0 + 64],
                                            r[:], None, MULT)
                o01s[qj] = o01
                o2s[qj] = o2

            oT01s = [None] * 4
            oT2s = [None] * 4

            def proj_t(qc, qj):
                """PE transposes + DVE copies for one q-tile (stage 1)."""
                tT = auxpool.tile([128, 512], F32, tag="aux",
                                  name=f"oT{qc}_{qj}")
                nc.tensor.transpose(tT[:, 0:128], o01s[qj][:], identF[:])
                nc.tensor.transpose(tT[0:64, 128:256], o2s[qj][:], identF[:])
                oT01 = opool.tile([128, 128], F32R, tag="oT01", bufs=4,
                                  name="oT01")
                oT2 = opool.tile([64, 128], F32R, tag="oT2", bufs=4,
                                 name="oT2")
                nc.vector.tensor_copy(oT01[:], tT[:, 0:128])
                nc.vector.tensor_copy(oT2[:], tT[0:64, 128:256])
                oT01s[qj] = oT01
                oT2s[qj] = oT2

            def proj_p(qc, qj):
                """proj matmuls + y out for one q-tile (stage 2)."""
                qrows = slice(qc * 512 + qj * 128, qc * 512 + (qj + 1) * 128)
                oT01, oT2 = oT01s[qj], oT2s[qj]
                ya = auxpool.tile([128, 512], F32, tag="aux",
                                  name=f"ya{qc}_{qj}")
                nc.tensor.matmul(ya[:], oT01[:], wp01_r[:, 0:512],
                                 start=True, stop=False)
                nc.tensor.matmul(ya[:], oT2[:], wp2_r[:, 0:512],
                                 start=False, stop=True)
                yb = auxpool.tile([128, 512], F32, tag="aux",
                                  name=f"yb{qc}_{qj}")
                nc.tensor.matmul(yb[:, 0:256], oT01[:], wp01_r[:, 512:768],
                                 start=True, stop=False)
                nc.tensor.matmul(yb[:, 0:256], oT2[:], wp2_r[:, 512:768],
                                 start=False, stop=True)
                y_sb = ypool.tile([128, C], F32, tag="y", name="ysb")
                nc.vector.tensor_copy(y_sb[:, 0:512], ya[:])
                nc.vector.tensor_copy(y_sb[:, 512:768], yb[:, 0:256])
                nc.sync.dma_start(out=y_d[qrows, :], in_=y_sb[:])

            # ---------------- schedule ----------------
            # per-round filler work keeps PE bursts small so the exp
            # stream on ACT never starves; all pieces are ~1us or less.
            def k1(ch):
                kq_block(ch, 1, kT01, 1)

            def k3(ch):
                kq_block(ch, 3, kT22, 3)

            def q0(ch):
                kq_block(ch, 0, qT01, 0)

            def q2(ch):
                kq_block(ch, 2, qT22, 2)

            def run_fillers(items):
                for f in items:
                    f()

            chunk_dma(0)
            chunk_dma(1)
            k1(0)
            q0(0)
            q0(1)
            bulk_dmas()
            # Filler plan per (qc, round 0..11). Deadlines: k1(ch) before
            # pair round ch; k3(2i),k3(2i+1) before h2 round i; q0/q2
            # slabs before their qc's pair/h2 rounds; v before pv pieces;
            # h2-chains 2,3 + norm of qc-1 before this qc's PV overwrites
            # acc (rounds 0-1).  pv pieces per qc: pair 0..3 at h2 rounds,
            # h2 chains 0,1 at qc end, chains 2,3 early next qc.
            prev = {"pts": None}

            def qc0_fill(j):
                if j < 7:
                    k1(j + 1)
                v_block(2 * j)
                if j == 4:
                    k3(0)
                if j == 5:
                    k3(1)
                if j == 6:
                    k3(2)
                    k3(3)
                if j == 7:
                    q2(0)
                    q2(1)

            for qc in range(NQC):
                ptsA, ptsB, hA, hB = [], [], [], []
                for j in range(8):
                    a, b = scores_pair(qc, j)
                    ptsA.append(a)
                    ptsB.append(b)
                    if qc == 0:
                        qc0_fill(j)
                    else:
                        pA0, pB0, hA0, hB0 = prev["pts"]
                        if j == 0:
                            pv_h2_chain(hA0, hB0, 2)
                            norm_qj(qc - 1, 0)
                            norm_qj(qc - 1, 1)
                        elif j == 1:
                            pv_h2_chain(hA0, hB0, 3)
                            norm_qj(qc - 1, 2)
                            norm_qj(qc - 1, 3)
                            proj_t(qc - 1, 0)
                        elif j in (2, 3, 4):
                            proj_p(qc - 1, j - 2)
                            proj_t(qc - 1, j - 1)
                        elif j == 5:
                            proj_p(qc - 1, 3)
                        elif j == 6:
                            q2(2 * qc)
                        elif j == 7:
                            q2(2 * qc + 1)
                for i in range(4):
                    a, b = scores_h2(qc, i)
                    hA.append(a)
                    hB.append(b)
                    if qc == 0:
                        if i == 0:
                            k3(4)
                            k3(5)
                        elif i == 1:
                            k3(6)
                            k3(7)
                            pv_pair_piece(ptsA, ptsB, 0)
                        elif i == 2:
                            pv_pair_piece(ptsA, ptsB, 1)
                            pv_pair_piece(ptsA, ptsB, 2)
                        else:
                            pv_pair_piece(ptsA, ptsB, 3)
                            q0(2)
                            q0(3)
                    else:
                        pv_pair_piece(ptsA, ptsB, i)
                        if qc < NQC - 1:
                            if i == 2:
                                q0(2 * qc + 2)
                            elif i == 3:
                                q0(2 * qc + 3)
                pv_h2_chain(hA, hB, 0)
                pv_h2_chain(hA, hB, 1)
                prev["pts"] = (ptsA, ptsB, hA, hB)
            # tail: per-qj pipeline to keep PE dense and DVE off the
            # critical path (norm(qj) ready one step before its T/P)
            pA0, pB0, hA0, hB0 = prev["pts"]
            qn = NQC - 1
            pv_h2_chain(hA0, hB0, 2)
            norm_qj(qn, 0)
            norm_qj(qn, 1)
            proj_t(qn, 0)
            pv_h2_chain(hA0, hB0, 3)
            norm_qj(qn, 2)
            norm_qj(qn, 3)
            proj_t(qn, 1)
            proj_p(qn, 0)
            proj_t(qn, 2)
            proj_p(qn, 1)
            proj_t(qn, 3)
            proj_p(qn, 2)
            proj_p(qn, 3)

    nc.compile()
    return nc


def make_in_maps(x, w_qkv, b_qkv, w_proj):
    """Per-core input dicts. Core c: batch c//4, heads 3*(c%4)+[0..2]."""
    x = np.asarray(x, np.float32)
    w_qkv = np.asarray(w_qkv, np.float32)
    b_qkv = np.asarray(b_qkv, np.float32)
    w_proj = np.asarray(w_proj, np.float32)
    bf = ml_dtypes.bfloat16
    q = lambda h: w_qkv[:, h * 64:(h + 1) * 64]
    k = lambda h: w_qkv[:, C + h * 64:C + (h + 1) * 64]
    v = lambda h: w_qkv[:, 2 * C + h * 64:2 * C + (h + 1) * 64]
    qb = lambda h: b_qkv[h * 64:(h + 1) * 64]
    kb = lambda h: b_qkv[C + h * 64:C + (h + 1) * 64]
    vb = lambda h: b_qkv[2 * C + h * 64:2 * C + (h + 1) * 64]
    in_maps = []
    for c in range(NCORES):
        b = c // 4
        h0 = 3 * (c % 4)
        h1, h2 = h0 + 1, h0 + 2
        w_pack = np.concatenate(
            [q(h0), q(h1), k(h0), k(h1), q(h2), q(h2), k(h2), k(h2)], axis=1)
        bias = np.concatenate(
            [qb(h0), qb(h1), kb(h0), kb(h1), qb(h2), qb(h2), kb(h2), kb(h2)])
        bq_pack = bias.reshape(4, 128).T.copy()
        wv_pack = np.concatenate([v(h0), v(h1), v(h2)], axis=1)
        vb_pack = np.broadcast_to(
            np.concatenate([vb(h0), vb(h1), vb(h2)]), (128, 192))
        wp01 = np.concatenate(
            [w_proj[h0 * 64:(h0 + 1) * 64], w_proj[h1 * 64:(h1 + 1) * 64]],
            axis=0)
        wp2 = w_proj[h2 * 64:(h2 + 1) * 64]
        # host transpose, chunk-major: xt[ch, p, ct*256+n'] = x[b][ch*256+n', ct*128+p]
        xt = np.ascontiguousarray(
            x[b].reshape(NCH, 256, CT, 128).transpose(0, 3, 2, 1).reshape(
                NCH, 128, CT * 256)).astype(bf)
        in_maps.append({
            "xt": xt,
            "w": np.ascontiguousarray(w_pack).astype(bf),
            "wv": np.ascontiguousarray(wv_pack).astype(bf),
            "bq": np.ascontiguousarray(bq_pack, np.float32),
            "vb": np.ascontiguousarray(vb_pack, np.float32),
            "wp01": np.ascontiguousarray(wp01, np.float32),
            "wp2": np.ascontiguousarray(wp2, np.float32),
        })
    return in_maps


_NC_CACHE = []


def _get_program():
    if not _NC_CACHE:
        _NC_CACHE.append(build_program())
    return _NC_CACHE[0]


def run(inputs, trace=False, **kw):
    nc = _get_program()
    in_maps = make_in_maps(inputs["x"], inputs["w_qkv"], inputs["b_qkv"],
                           inputs["w_proj"])
    res = run_bass_kernel_spmd(nc, in_maps, list(range(NCORES)), trace=trace, **kw)
    b_proj = np.asarray(inputs["b_proj"], np.float32)
    out = np.zeros((B, N, C), np.float32)
    for c in range(NCORES):
        out[c // 4] += res.results[c]["y"]
    out += b_proj[None, None, :]
    return out.astype(np.float32), res


def kernel(**inputs):
    out, _ = run(inputs)
    return out
